# revision 17
# baseline (speedup 1.0000x reference)
"""Trainium2 Bass kernel for nn_AttentionBlock (B=4, T=2048, C=K=V=1024).

Self-contained: builds one SPMD Bass/Tile program, runs it on 8 NeuronCores
via run_bass_kernel_spmd, and reassembles the full output on the host.

Math (matches the reference):
  q/k/v = x @ W + b ; logits[b,t,s] = q.k, causal mask s<=t ;
  probs = softmax(logits/sqrt(K), axis=t)   # over the QUERY axis
  read = probs @ v ; out = concat(x, read, axis=2)

Sharding (zero-collective): core = 2*b + h owns batch b and the interleaved
key/value tiles sigma = 2*i + h (interleaving balances the causal triangle).
Because the softmax normalizes over the query axis t and each core has ALL
queries for its own key columns, the softmax is fully core-local. Each core
computes q in full, k/v only for its own columns, exp scores et[s_own, t],
and an additive partial read_h[t, v] = et^T @ (v_own * dinv). The host sums
the two partials per batch and concatenates x.

Numerics (fp8): the q/k/score path runs in fp8 e4m3 with DoubleRow matmuls
(2 contraction rows per PE pass -> 2x FLOP rate; measured 216 ns per
512-col matmul = 1 cyc/col at 2.37 GHz); the v/read path stays bf16
because probs/v quantization error transfers directly to the output
(simulated: ABC-fp8 + DE-bf16 = 1.1e-2 relmax vs the 2e-2 gate; all-fp8 =
4.5e-2). All accumulation is f32 PSUM. Weights quantize to fp8 at natural
scale (e4m3 subnormals cover the 0.02-std weights). read partials return
bf16 (halves output DMA).

Engine/DMA layout (from NTFF traces of earlier revisions):
 - Phase A is sb-outer so compute streams behind the xt chunk DMAs.
 - All streamed inputs are host-packed chunk-major as [chunk][ci][co][s]:
   each DMA chunk is linear in DRAM and lands as one contiguous ~4 KB
   run per SBUF partition. The naive [C, T] layout produced 256 B SWDGE
   packets that capped the early input stream at ~50 GB/s and stalled
   phase A's start by ~7 us.
 - Exp row-sums are whole-row DVE reduces (no ACT accum_out: its fused
   ACTIVATION_READ_ACCUMULATOR serialized ~290 ns per chunk); the
   softmax 1/rowsum is folded into v_own via an ACT Copy(scale=dinv)
   during phase D (ACT is idle there; gpsimd elementwise measured
   ~17 us per [128,1024] op - unusable).
 - PSUM->SBUF bias adds all on DVE (~830 ns per [128,512] f32; gpsimd
   cannot read PSUM).
 - Phase E's last row-group writes 256-col output chunks to shorten the
   post-matmul copy+DMA tail.

Per-core phases (all at engine roofline on the NTFF trace):
  A. qt [k, 2048] = wq.T @ x^T + bq            fp8 DoubleRow (ci pairs)
  B. kt_own [k, 1024] = wk.T @ xst + bk        fp8 DoubleRow (ci pairs)
  C. et_i = exp((kt_i.T @ qt + mask)/32)       fp8 DoubleRow (ko pairs)
     dsum_i = rowsum(et_i) via DVE; dinv = 1/dsum (DVE reciprocal)
  D. v_own [1024, v] = (xst.T @ wv + bv) * dinv   bf16
  E. read_partial[t, v] = sum_i et_i.T @ v_own_i -> DRAM (bf16)
"""

from contextlib import ExitStack

import numpy as np

import concourse.mybir as mybir
import concourse.tile as tile
from concourse import bacc
from concourse._compat import with_exitstack

P = 128
B = 4
T = 2048
C = 1024
KD = 1024
VD = 1024
NCO = C // P
NKO = KD // P
NCP = NCO // 2  # ci pairs (fp8 DoubleRow contraction granularity)
NKP = NKO // 2  # ko pairs
NI = 8
SOWN = NI * P
SQRT_K = 32.0
F32 = mybir.dt.float32
CD = mybir.dt.bfloat16
F8 = mybir.dt.float8e4
DR = mybir.MatmulPerfMode.DoubleRow
SB = 512
NBLK = T // SB
NSB = SOWN // SB


@with_exitstack
def attn_body(ctx: ExitStack, tc, io):
    nc = tc.nc
    # chunk-major DRAM layouts (host pre-packed as [chunk][ci][co][cols]);
    # each [P, NCO*cols] slice below is one fully-linear DRAM chunk.
    xt = io["xt"].ap()
    xst8 = io["xst8"].ap()
    xst = io["xst"].ap()
    wq = io["wq"].ap()
    wk = io["wk"].ap()
    wv = io["wv"].ap().rearrange("(co ci) k -> ci co k", ci=P)

    const = ctx.enter_context(tc.tile_pool(name="const", bufs=1))
    bk_sb = const.tile([P, NKO], F32)
    bq_sb = const.tile([P, NKO], F32)
    mask_sb = const.tile([P, NI, 2 * P], F32)

    psum = ctx.enter_context(tc.tile_pool(name="psum", bufs=8, space="PSUM"))

    wp = ctx.enter_context(tc.tile_pool(name="wp", bufs=1, side="right"))
    wq_sb = wp.tile([P, NKO, NCO, P], F8, name="wq_sb")
    wk_sb = wp.tile([P, NKO, NCO, P], F8, name="wk_sb")
    wv_sb = wp.tile([P, NCO, VD], CD, name="wv_sb")
    bv_sb = wp.tile([P, VD], F32, name="bv_sb")
    xstp = ctx.enter_context(tc.tile_pool(name="xstp", bufs=1, side="right"))
    xst8_sb = xstp.tile([P, NSB, NCO, SB], F8)
    xst_sb = xstp.tile([P, NSB, NCO, SB], CD)

    # ---------------- phase A: qt = wq.T @ xt + bq (full t) --------------
    # sb-outer so compute streams behind the xt chunk DMAs: the first
    # matmul group needs only xt chunk 0 (sync q) + wq slice 0 (gpsimd q).
    qtp = ctx.enter_context(tc.tile_pool(name="qtp", bufs=1))
    qt = qtp.tile([P, NKO, T], F8, tag="qt")
    xtp_cm = tc.tile_pool(name="xtp", bufs=1, side="right")
    xtp = xtp_cm.__enter__()
    xt_sb = xtp.tile([P, NBLK, NCO, SB], F8)
    for sb in range(NBLK):
        nc.sync.dma_start(xt_sb[:, sb], xt[sb * P : (sb + 1) * P])
    for ko in range(NKO):
        nc.gpsimd.dma_start(wq_sb[:, ko], wq[ko * P : (ko + 1) * P])
    nc.gpsimd.dma_start(bq_sb[:], io["bq2"].ap())
    nc.gpsimd.dma_start(bk_sb[:], io["bk2"].ap())
    nc.gpsimd.dma_start(mask_sb[:], io["maskbias"].ap())
    for sb in range(NBLK):
        for ko in range(NKO):
            ps = psum.tile([P, SB], F32, tag="ps", name=f"psA{sb}_{ko}")
            for cp in range(NCP):
                nc.tensor.matmul(
                    ps[:],
                    wq_sb[:, ko, 2 * cp : 2 * cp + 2, :],
                    xt_sb[:, sb, 2 * cp : 2 * cp + 2, :],
                    start=(cp == 0),
                    stop=(cp == NCP - 1),
                    perf_mode=DR,
                )
            nc.vector.tensor_add(
                qt[:, ko, sb * SB : (sb + 1) * SB],
                ps[:],
                bq_sb[:, ko : ko + 1].to_broadcast((P, SB)),
            )
    xtp_cm.__exit__(None, None, None)

    # loads for phases B and D: xst8 behind xt on the sync queue; wk and
    # the bf16 D inputs behind wq on the gpsimd queue.
    for sb in range(NSB):
        nc.sync.dma_start(xst8_sb[:, sb], xst8[sb * P : (sb + 1) * P])
    for ko in range(NKO):
        nc.gpsimd.dma_start(wk_sb[:, ko], wk[ko * P : (ko + 1) * P])
    for sb in range(NSB):
        nc.gpsimd.dma_start(xst_sb[:, sb], xst[sb * P : (sb + 1) * P])
    nc.gpsimd.dma_start(wv_sb[:], wv)
    nc.gpsimd.dma_start(bv_sb[:], io["bv2"].ap())

    # ---------------- phase B: kt_own = wk.T @ xst + bk ----------------
    ktp = ctx.enter_context(tc.tile_pool(name="ktp", bufs=1))
    kt = ktp.tile([P, NKO, SOWN], F8, tag="kt")
    for ko in range(NKO):
        pss = [
            psum.tile([P, SB], F32, tag="ps", name=f"psB{ko}_{sb}")
            for sb in range(NSB)
        ]
        for cp in range(NCP):
            for sb in range(NSB):
                nc.tensor.matmul(
                    pss[sb][:],
                    wk_sb[:, ko, 2 * cp : 2 * cp + 2, :],
                    xst8_sb[:, sb, 2 * cp : 2 * cp + 2, :],
                    start=(cp == 0),
                    stop=(cp == NCP - 1),
                    perf_mode=DR,
                )
        for sb in range(NSB):
            nc.vector.tensor_add(
                kt[:, ko, sb * SB : (sb + 1) * SB],
                pss[sb][:],
                bk_sb[:, ko : ko + 1].to_broadcast((P, SB)),
            )

    # ------ phase C: et_i = exp((kt_i.T @ qt + mask)/32) ------
    # Stationary-major over ko pairs with multi-PSUM chunks. ACT does only
    # the exp; the row sum is one whole-row DVE reduce per block, and the
    # normalizer is folded into v_own during phase D.
    etp = ctx.enter_context(tc.tile_pool(name="etp", bufs=1, side="right"))
    et = etp.tile([P, NI, T], CD, tag="et")
    dsum = const.tile([P, NI], F32, name="dsum")
    dinv = const.tile([P, NI], F32, name="dinv")
    for i in range(NI):
        tstart = 2 * i * P
        chunks = []
        t0 = tstart
        while t0 < T:
            w = min(SB, T - t0)
            chunks.append((t0, w))
            t0 += w
        pss = [
            psum.tile([P, SB], F32, tag="ps", name=f"psC{i}_{c}")
            for c in range(len(chunks))
        ]
        for kp in range(NKP):
            for c, (t0, w) in enumerate(chunks):
                nc.tensor.matmul(
                    pss[c][:, :w],
                    kt[:, 2 * kp : 2 * kp + 2, i * P : (i + 1) * P],
                    qt[:, 2 * kp : 2 * kp + 2, t0 : t0 + w],
                    start=(kp == 0),
                    stop=(kp == NKP - 1),
                    perf_mode=DR,
                )
        nc.vector.tensor_add(
            pss[0][:, : 2 * P], pss[0][:, : 2 * P], mask_sb[:, i, :]
        )
        for c, (t0, w) in enumerate(chunks):
            nc.scalar.activation(
                et[:, i, t0 : t0 + w],
                pss[c][:, :w],
                mybir.ActivationFunctionType.Exp,
                scale=1.0 / SQRT_K,
            )
        nc.vector.tensor_reduce(
            dsum[:, i : i + 1],
            et[:, i, tstart:],
            axis=mybir.AxisListType.X,
            op=mybir.AluOpType.add,
        )
        nc.vector.reciprocal(dinv[:, i : i + 1], dsum[:, i : i + 1])

    # -------- phase D: v_own = (xst.T @ wv + bv) * dinv[own row] --------
    vop = ctx.enter_context(tc.tile_pool(name="vop", bufs=1))
    v_own = vop.tile([P, NI, VD], CD)
    for jl in range(NI):
        pss = [
            psum.tile([P, SB], F32, tag="ps", name=f"psD{jl}_{vb}")
            for vb in range(VD // SB)
        ]
        for ci in range(NCO):
            for vb in range(VD // SB):
                nc.tensor.matmul(
                    pss[vb][:],
                    xst_sb[:, jl // 4, ci, (jl % 4) * P : (jl % 4 + 1) * P],
                    wv_sb[:, ci, vb * SB : (vb + 1) * SB],
                    start=(ci == 0),
                    stop=(ci == NCO - 1),
                )
        for vb in range(VD // SB):
            nc.vector.tensor_add(
                v_own[:, jl, vb * SB : (vb + 1) * SB],
                pss[vb][:],
                bv_sb[:, vb * SB : (vb + 1) * SB],
            )
        nc.scalar.activation(
            v_own[:, jl, :],
            v_own[:, jl, :],
            mybir.ActivationFunctionType.Copy,
            scale=dinv[:, jl : jl + 1],
        )

    # ------------- phase E: read_partial = sum_i et_i.T @ v_i -------------
    read_out = io["read_out"].ap()
    with tc.tile_pool(name="rout", bufs=8) as rout:
        for g in range(T // P):
            ni = g // 2 + 1
            pss = [
                psum.tile([P, SB], F32, tag="ps", name=f"psE{g}_{vb}")
                for vb in range(VD // SB)
            ]
            for i in range(ni):
                for vb in range(VD // SB):
                    nc.tensor.matmul(
                        pss[vb][:],
                        et[:, i, g * P : (g + 1) * P],
                        v_own[:, i, vb * SB : (vb + 1) * SB],
                        start=(i == 0),
                        stop=(i == ni - 1),
                    )
            # last row-group: 256-col output chunks so the final
            # copy+DMA tail after the last matmul is as short as possible
            W = 2 * P if g == T // P - 1 else SB
            for vb in range(VD // W):
                ro = rout.tile([P, W], CD, tag="rout")
                src = pss[vb * W // SB][:, vb * W % SB : vb * W % SB + W]
                if (2 * g + vb) % 2 == 0:
                    nc.scalar.copy(ro[:], src)
                else:
                    nc.vector.tensor_copy(ro[:], src)
                # last row-group: alternate queues so the ~600 ns DMA
                # enqueues overlap instead of serializing on one queue
                q = nc.sync if (vb if g == T // P - 1 else 2 * g + vb) % 2 == 0 else nc.gpsimd
                q.dma_start(
                    read_out[g * P : (g + 1) * P, vb * W : (vb + 1) * W],
                    ro[:],
                )


def _build_nc(num_devices=8):
    nc = bacc.Bacc(
        "TRN2", target_bir_lowering=False, debug=False, num_devices=num_devices
    )
    io = {}
    io["xt"] = nc.dram_tensor("xt", [NBLK * P, NCO * SB], F8, kind="ExternalInput")
    io["xst8"] = nc.dram_tensor(
        "xst8", [NSB * P, NCO * SB], F8, kind="ExternalInput"
    )
    io["xst"] = nc.dram_tensor(
        "xst", [NSB * P, NCO * SB], CD, kind="ExternalInput"
    )
    for w in ("wq", "wk"):
        io[w] = nc.dram_tensor(w, [NKO * P, NCO * P], F8, kind="ExternalInput")
    io["wv"] = nc.dram_tensor("wv", [C, KD], CD, kind="ExternalInput")
    io["bk2"] = nc.dram_tensor("bk2", [P, NKO], F32, kind="ExternalInput")
    io["bq2"] = nc.dram_tensor("bq2", [P, NKO], F32, kind="ExternalInput")
    io["bv2"] = nc.dram_tensor("bv2", [P, VD], F32, kind="ExternalInput")
    io["maskbias"] = nc.dram_tensor(
        "maskbias", [P, NI * 2 * P], F32, kind="ExternalInput"
    )
    io["read_out"] = nc.dram_tensor(
        "read_out", [T, VD], CD, kind="ExternalOutput"
    )
    with tile.TileContext(nc) as tc:
        attn_body(tc, io)
    nc.compile()
    return nc


def _own_cols(h):
    idx = []
    for j in range(NI):
        g = 2 * j + (h ^ (j & 1))
        idx.extend(range(g * P, (g + 1) * P))
    return np.array(idx)


def _pack(a, w):
    """[C, Wtot] -> [(Wtot//w)*P, NCO*w]: chunk-major, each chunk laid out
    [ci][co][s] so a chunk DMA is linear in DRAM and writes one contiguous
    run per SBUF partition."""
    nb = a.shape[1] // w
    return np.ascontiguousarray(
        a.reshape(NCO, P, nb, w).transpose(2, 1, 0, 3).reshape(nb * P, NCO * w)
    )


def _make_in_maps(x, Wq, bq, Wk, bk, Wv, bv):
    import ml_dtypes

    bf16 = ml_dtypes.bfloat16
    f8 = ml_dtypes.float8_e4m3
    x = np.asarray(x, np.float32)
    Wq8, Wk8 = (np.asarray(w, np.float32).astype(f8) for w in (Wq, Wk))
    Wvb = np.ascontiguousarray(np.asarray(Wv, np.float32).astype(bf16))
    bq, bk, bv = (np.asarray(v, np.float32) for v in (bq, bk, bv))

    sr = np.arange(P)[:, None]
    tcc = np.arange(P)[None, :]
    tri = np.where(tcc >= sr, 0.0, -1e9).astype(np.float32)
    masks = {}
    for h in (0, 1):
        m = np.zeros((P, NI, 2 * P), np.float32)
        for j in range(NI):
            if (h ^ (j & 1)) == 0:  # own block is the early one (pos 2j)
                m[:, j, :P] = tri
            else:
                m[:, j, :P] = -1e9
                m[:, j, P:] = tri
        masks[h] = m.reshape(P, NI * 2 * P)

    bk2 = np.ascontiguousarray(bk.reshape(NKO, P).T)
    bq2 = np.ascontiguousarray(bq.reshape(NKO, P).T)
    bv2 = np.ascontiguousarray(np.broadcast_to(bv[None, :], (P, VD)))

    Wq8c, Wk8c = (_pack(w, P) for w in (Wq8, Wk8))
    in_maps = []
    for core in range(8):
        b, h = core // 2, core % 2
        xt_b = np.ascontiguousarray(x[b].T)
        own = _own_cols(h)
        in_maps.append(
            {
                "xt": _pack(xt_b.astype(f8), SB),
                "xst8": _pack(xt_b[:, own].astype(f8), SB),
                "xst": _pack(xt_b[:, own].astype(bf16), SB),
                "wq": Wq8c,
                "wk": Wk8c,
                "wv": Wvb,
                "bk2": bk2,
                "bq2": bq2,
                "bv2": bv2,
                "maskbias": masks[h],
            }
        )
    return in_maps


def _assemble_output(x, results):
    x = np.asarray(x, np.float32)
    out = np.empty((x.shape[0], T, C + VD), np.float32)
    out[:, :, :C] = x
    for b in range(x.shape[0]):
        out[b, :, C:] = results[2 * b]["read_out"].astype(np.float32) + results[
            2 * b + 1
        ]["read_out"].astype(np.float32)
    return out


_NC_CACHE = None


def _build():
    global _NC_CACHE
    if _NC_CACHE is None:
        _NC_CACHE = _build_nc(num_devices=8)
    return _NC_CACHE


def kernel(x, Wq, bq, Wk, bk, Wv, bv):
    from concourse.bass_utils import run_bass_kernel_spmd

    nc = _build()
    in_maps = _make_in_maps(x, Wq, bq, Wk, bk, Wv, bv)
    res = run_bass_kernel_spmd(nc, in_maps, core_ids=list(range(8)))
    return _assemble_output(x, res.results)


# revision 18
# speedup vs baseline: 1.1925x; 1.1925x over previous
"""Trainium2 Bass kernel for nn_AttentionBlock (B=4, T=2048, C=K=V=1024).

Self-contained: builds one SPMD Bass/Tile program, runs it on 8 NeuronCores
via run_bass_kernel_spmd, and reassembles the full output on the host.

Math (matches the reference):
  q/k/v = x @ W + b ; logits[b,t,s] = q.k, causal mask s<=t ;
  probs = softmax(logits/sqrt(K), axis=t)   # over the QUERY axis
  read = probs @ v ; out = concat(x, read, axis=2)

Sharding (zero-collective): core = 2*b + h owns batch b and the interleaved
key/value tiles sigma = 2*i + h (interleaving balances the causal triangle).
Because the softmax normalizes over the query axis t and each core has ALL
queries for its own key columns, the softmax is fully core-local. Each core
computes q in full, k/v only for its own columns, exp scores et[s_own, t],
and an additive partial read_h[t, v] = et^T @ (v_own * dinv). The host sums
the two partials per batch and concatenates x.

Numerics (fp8): the q/k/score path runs in fp8 e4m3 with DoubleRow matmuls
(2 contraction rows per PE pass -> 2x FLOP rate; measured 216 ns per
512-col matmul = 1 cyc/col at 2.37 GHz); the v/read path stays bf16
because probs/v quantization error transfers directly to the output
(simulated: ABC-fp8 + DE-bf16 = 1.1e-2 relmax vs the 2e-2 gate; all-fp8 =
4.5e-2). All accumulation is f32 PSUM. Weights quantize to fp8 at natural
scale (e4m3 subnormals cover the 0.02-std weights). read partials return
bf16 (halves output DMA).

Engine/DMA layout (from NTFF traces of earlier revisions):
 - Phase A is sb-outer so compute streams behind the xt chunk DMAs.
 - All streamed inputs are host-packed chunk-major as [chunk][ci][co][s]:
   each DMA chunk is linear in DRAM and lands as one contiguous ~4 KB
   run per SBUF partition. The naive [C, T] layout produced 256 B SWDGE
   packets that capped the early input stream at ~50 GB/s and stalled
   phase A's start by ~7 us.
 - Exp row-sums are whole-row DVE reduces (no ACT accum_out: its fused
   ACTIVATION_READ_ACCUMULATOR serialized ~290 ns per chunk); the
   softmax 1/rowsum is folded into v_own via an ACT Copy(scale=dinv)
   during phase D (ACT is idle there; gpsimd elementwise measured
   ~17 us per [128,1024] op - unusable).
 - PSUM->SBUF bias adds all on DVE (~830 ns per [128,512] f32; gpsimd
   cannot read PSUM).
 - Phase E's last row-group writes 256-col output chunks to shorten the
   post-matmul copy+DMA tail.

Per-core phases (all at engine roofline on the NTFF trace):
  A. qt [k, 2048] = wq.T @ x^T + bq            fp8 DoubleRow (ci pairs)
  B. kt_own [k, 1024] = wk.T @ xst + bk        fp8 DoubleRow (ci pairs)
  C. et_i = exp((kt_i.T @ qt + mask)/32)       fp8 DoubleRow (ko pairs)
     dsum_i = rowsum(et_i) via DVE; dinv = 1/dsum (DVE reciprocal)
  D. v_own [1024, v] = (xst.T @ wv + bv) * dinv   bf16
  E. read_partial[t, v] = sum_i et_i.T @ v_own_i -> DRAM (bf16)
"""

from contextlib import ExitStack

import numpy as np

import concourse.mybir as mybir
import concourse.tile as tile
from concourse import bacc
from concourse._compat import with_exitstack

P = 128
B = 4
T = 2048
C = 1024
KD = 1024
VD = 1024
NCO = C // P
NKO = KD // P
NCP = NCO // 2  # ci pairs (fp8 DoubleRow contraction granularity)
NKP = NKO // 2  # ko pairs
NI = 8
SOWN = NI * P
SQRT_K = 32.0
F32 = mybir.dt.float32
CD = mybir.dt.bfloat16
F8 = mybir.dt.float8e4
DR = mybir.MatmulPerfMode.DoubleRow
SB = 512
NBLK = T // SB
NSB = SOWN // SB


@with_exitstack
def attn_body(ctx: ExitStack, tc, io):
    nc = tc.nc
    # chunk-major DRAM layouts (host pre-packed as [chunk][ci][co][cols]);
    # each [P, NCO*cols] slice below is one fully-linear DRAM chunk.
    xt = io["xt"].ap()
    xst8 = io["xst8"].ap()
    xst = io["xst"].ap()
    wq = io["wq"].ap()
    wk = io["wk"].ap()
    wv = io["wv"].ap().rearrange("(co ci) k -> ci co k", ci=P)

    const = ctx.enter_context(tc.tile_pool(name="const", bufs=1))
    bk_sb = const.tile([P, NKO], F32)
    bq_sb = const.tile([P, NKO], F32)
    mask_sb = const.tile([P, 2 * P], F32)

    psum = ctx.enter_context(tc.tile_pool(name="psum", bufs=8, space="PSUM"))

    wp = ctx.enter_context(tc.tile_pool(name="wp", bufs=1, side="right"))
    wq_sb = wp.tile([P, NKO, NCO, P], F8, name="wq_sb")
    wk_sb = wp.tile([P, NKO, NCO, P], F8, name="wk_sb")
    wv_sb = wp.tile([P, NCO, VD], CD, name="wv_sb")
    bv_sb = wp.tile([P, VD], F32, name="bv_sb")
    xstp = ctx.enter_context(tc.tile_pool(name="xstp", bufs=1, side="right"))
    xst8_sb = xstp.tile([P, NSB, NCO, SB], F8)
    xst_sb = xstp.tile([P, NSB, NCO, SB], CD)

    # ---------------- phase A: qt = wq.T @ xt + bq (full t) --------------
    # sb-outer so compute streams behind the xt chunk DMAs: the first
    # matmul group needs only xt chunk 0 (sync q) + wq slice 0 (gpsimd q).
    qtp = ctx.enter_context(tc.tile_pool(name="qtp", bufs=1))
    qt = qtp.tile([P, NKO, T], F8, tag="qt")
    xtp_cm = tc.tile_pool(name="xtp", bufs=1, side="right")
    xtp = xtp_cm.__enter__()
    xt_sb = xtp.tile([P, NBLK, NCO, SB], F8)
    for sb in range(NBLK):
        nc.sync.dma_start(xt_sb[:, sb], xt[sb * P : (sb + 1) * P])
    for ko in range(NKO):
        nc.gpsimd.dma_start(wq_sb[:, ko], wq[ko * P : (ko + 1) * P])
    nc.gpsimd.dma_start(bq_sb[:], io["bq2"].ap())
    nc.gpsimd.dma_start(bk_sb[:], io["bk2"].ap())
    nc.gpsimd.dma_start(mask_sb[:], io["maskbias"].ap())
    for sb in range(NBLK):
        for ko in range(NKO):
            ps = psum.tile([P, SB], F32, tag="ps", name=f"psA{sb}_{ko}")
            for cp in range(NCP):
                nc.tensor.matmul(
                    ps[:],
                    wq_sb[:, ko, 2 * cp : 2 * cp + 2, :],
                    xt_sb[:, sb, 2 * cp : 2 * cp + 2, :],
                    start=(cp == 0),
                    stop=(cp == NCP - 1),
                    perf_mode=DR,
                )
            nc.vector.tensor_add(
                qt[:, ko, sb * SB : (sb + 1) * SB],
                ps[:],
                bq_sb[:, ko : ko + 1].to_broadcast((P, SB)),
            )
    xtp_cm.__exit__(None, None, None)

    # loads for phases B and D: xst8 behind xt on the sync queue; wk and
    # the bf16 D inputs behind wq on the gpsimd queue.
    for sb in range(NSB):
        nc.sync.dma_start(xst8_sb[:, sb], xst8[sb * P : (sb + 1) * P])
    for ko in range(NKO):
        nc.gpsimd.dma_start(wk_sb[:, ko], wk[ko * P : (ko + 1) * P])
    for sb in range(NSB):
        nc.gpsimd.dma_start(xst_sb[:, sb], xst[sb * P : (sb + 1) * P])
    nc.gpsimd.dma_start(wv_sb[:], wv)
    nc.gpsimd.dma_start(bv_sb[:], io["bv2"].ap())

    # ---------------- phase B: kt_own = wk.T @ xst + bk ----------------
    ktp = ctx.enter_context(tc.tile_pool(name="ktp", bufs=1))
    kt = ktp.tile([P, NKO, SOWN], F8, tag="kt")
    for ko in range(NKO):
        pss = [
            psum.tile([P, SB], F32, tag="ps", name=f"psB{ko}_{sb}")
            for sb in range(NSB)
        ]
        for cp in range(NCP):
            for sb in range(NSB):
                nc.tensor.matmul(
                    pss[sb][:],
                    wk_sb[:, ko, 2 * cp : 2 * cp + 2, :],
                    xst8_sb[:, sb, 2 * cp : 2 * cp + 2, :],
                    start=(cp == 0),
                    stop=(cp == NCP - 1),
                    perf_mode=DR,
                )
        for sb in range(NSB):
            nc.vector.tensor_add(
                kt[:, ko, sb * SB : (sb + 1) * SB],
                pss[sb][:],
                bk_sb[:, ko : ko + 1].to_broadcast((P, SB)),
            )

    # ------ phase C: et_i = exp((kt_i.T @ qt + mask)/32) ------
    # Stationary-major over ko pairs with multi-PSUM chunks. ACT does only
    # the exp; the row sum is one whole-row DVE reduce per block, and the
    # normalizer is folded into v_own during phase D.
    etp = ctx.enter_context(tc.tile_pool(name="etp", bufs=1, side="right"))
    et = etp.tile([P, NI, T], CD, tag="et")
    dsum = const.tile([P, NI], F32, name="dsum")
    dinv = const.tile([P, NI], F32, name="dinv")
    for i in range(NI):
        tstart = 2 * i * P
        chunks = []
        t0 = tstart
        while t0 < T:
            w = min(SB, T - t0)
            chunks.append((t0, w))
            t0 += w
        pss = [
            psum.tile([P, SB], F32, tag="ps", name=f"psC{i}_{c}")
            for c in range(len(chunks))
        ]
        for kp in range(NKP):
            for c, (t0, w) in enumerate(chunks):
                nc.tensor.matmul(
                    pss[c][:, :w],
                    kt[:, 2 * kp : 2 * kp + 2, i * P : (i + 1) * P],
                    qt[:, 2 * kp : 2 * kp + 2, t0 : t0 + w],
                    start=(kp == 0),
                    stop=(kp == NKP - 1),
                    perf_mode=DR,
                )
        nc.vector.tensor_add(pss[0][:, : 2 * P], pss[0][:, : 2 * P], mask_sb[:])
        for c, (t0, w) in enumerate(chunks):
            nc.scalar.activation(
                et[:, i, t0 : t0 + w],
                pss[c][:, :w],
                mybir.ActivationFunctionType.Exp,
                scale=1.0 / SQRT_K,
            )
        nc.vector.tensor_reduce(
            dsum[:, i : i + 1],
            et[:, i, tstart:],
            axis=mybir.AxisListType.X,
            op=mybir.AluOpType.add,
        )
        nc.vector.reciprocal(dinv[:, i : i + 1], dsum[:, i : i + 1])

    # -------- phase D: v_own = (xst.T @ wv + bv) * dinv[own row] --------
    vop = ctx.enter_context(tc.tile_pool(name="vop", bufs=1))
    v_own = vop.tile([P, NI, VD], CD)
    for jl in range(NI):
        pss = [
            psum.tile([P, SB], F32, tag="ps", name=f"psD{jl}_{vb}")
            for vb in range(VD // SB)
        ]
        for ci in range(NCO):
            for vb in range(VD // SB):
                nc.tensor.matmul(
                    pss[vb][:],
                    xst_sb[:, jl // 4, ci, (jl % 4) * P : (jl % 4 + 1) * P],
                    wv_sb[:, ci, vb * SB : (vb + 1) * SB],
                    start=(ci == 0),
                    stop=(ci == NCO - 1),
                )
        for vb in range(VD // SB):
            nc.vector.tensor_add(
                v_own[:, jl, vb * SB : (vb + 1) * SB],
                pss[vb][:],
                bv_sb[:, vb * SB : (vb + 1) * SB],
            )
        nc.scalar.activation(
            v_own[:, jl, :],
            v_own[:, jl, :],
            mybir.ActivationFunctionType.Copy,
            scale=dinv[:, jl : jl + 1],
        )

    # ------------- phase E: read_partial = sum_i et_i.T @ v_i -------------
    read_out = io["read_out"].ap()
    with tc.tile_pool(name="rout", bufs=8) as rout:
        for g in range(T // P):
            ni = g // 2 + 1
            pss = [
                psum.tile([P, SB], F32, tag="ps", name=f"psE{g}_{vb}")
                for vb in range(VD // SB)
            ]
            for i in range(ni):
                for vb in range(VD // SB):
                    nc.tensor.matmul(
                        pss[vb][:],
                        et[:, i, g * P : (g + 1) * P],
                        v_own[:, i, vb * SB : (vb + 1) * SB],
                        start=(i == 0),
                        stop=(i == ni - 1),
                    )
            # last row-group: 256-col output chunks so the final
            # copy+DMA tail after the last matmul is as short as possible
            W = 2 * P if g == T // P - 1 else SB
            for vb in range(VD // W):
                ro = rout.tile([P, W], CD, tag="rout")
                src = pss[vb * W // SB][:, vb * W % SB : vb * W % SB + W]
                if (2 * g + vb) % 2 == 0:
                    nc.scalar.copy(ro[:], src)
                else:
                    nc.vector.tensor_copy(ro[:], src)
                # last row-group: alternate queues so the ~600 ns DMA
                # enqueues overlap instead of serializing on one queue
                q = (
                    nc.sync
                    if (vb if g == T // P - 1 else 2 * g + vb) % 2 == 0
                    else nc.gpsimd
                )
                q.dma_start(
                    read_out[g * P : (g + 1) * P, vb * W : (vb + 1) * W],
                    ro[:],
                )


def _build_nc(num_devices=8):
    nc = bacc.Bacc(
        "TRN2", target_bir_lowering=False, debug=False, num_devices=num_devices
    )
    io = {}
    io["xt"] = nc.dram_tensor("xt", [NBLK * P, NCO * SB], F8, kind="ExternalInput")
    io["xst8"] = nc.dram_tensor(
        "xst8", [NSB * P, NCO * SB], F8, kind="ExternalInput"
    )
    io["xst"] = nc.dram_tensor(
        "xst", [NSB * P, NCO * SB], CD, kind="ExternalInput"
    )
    for w in ("wq", "wk"):
        io[w] = nc.dram_tensor(w, [NKO * P, NCO * P], F8, kind="ExternalInput")
    io["wv"] = nc.dram_tensor("wv", [C, KD], CD, kind="ExternalInput")
    io["bk2"] = nc.dram_tensor("bk2", [P, NKO], F32, kind="ExternalInput")
    io["bq2"] = nc.dram_tensor("bq2", [P, NKO], F32, kind="ExternalInput")
    io["bv2"] = nc.dram_tensor("bv2", [P, VD], F32, kind="ExternalInput")
    io["maskbias"] = nc.dram_tensor(
        "maskbias", [P, 2 * P], F32, kind="ExternalInput"
    )
    io["read_out"] = nc.dram_tensor(
        "read_out", [T, VD], CD, kind="ExternalOutput"
    )
    with tile.TileContext(nc) as tc:
        attn_body(tc, io)
    nc.compile()
    return nc


def _own_cols(h):
    idx = []
    for i in range(NI):
        g = 2 * i + h
        idx.extend(range(g * P, (g + 1) * P))
    return np.array(idx)


def _pack(a, w):
    """[C, Wtot] -> [(Wtot//w)*P, NCO*w]: chunk-major, each chunk laid out
    [ci][co][s] so a chunk DMA is linear in DRAM and writes one contiguous
    run per SBUF partition."""
    nb = a.shape[1] // w
    return np.ascontiguousarray(
        a.reshape(NCO, P, nb, w).transpose(2, 1, 0, 3).reshape(nb * P, NCO * w)
    )


def _make_in_maps(x, Wq, bq, Wk, bk, Wv, bv):
    import ml_dtypes

    bf16 = ml_dtypes.bfloat16
    f8 = ml_dtypes.float8_e4m3
    x = np.asarray(x, np.float32)
    Wq8, Wk8 = (np.asarray(w, np.float32).astype(f8) for w in (Wq, Wk))
    Wvb = np.ascontiguousarray(np.asarray(Wv, np.float32).astype(bf16))
    bq, bk, bv = (np.asarray(v, np.float32) for v in (bq, bk, bv))

    sr = np.arange(P)[:, None]
    tcc = np.arange(P)[None, :]
    tri = np.where(tcc >= sr, 0.0, -1e9).astype(np.float32)
    masks = {}
    for h in (0, 1):
        m = np.zeros((P, 2 * P), np.float32)
        if h == 0:
            m[:, :P] = tri
        else:
            m[:, :P] = -1e9
            m[:, P:] = tri
        masks[h] = m

    bk2 = np.ascontiguousarray(bk.reshape(NKO, P).T)
    bq2 = np.ascontiguousarray(bq.reshape(NKO, P).T)
    bv2 = np.ascontiguousarray(np.broadcast_to(bv[None, :], (P, VD)))

    Wq8c, Wk8c = (_pack(w, P) for w in (Wq8, Wk8))
    in_maps = []
    for core in range(8):
        b, h = core // 2, core % 2
        xt_b = np.ascontiguousarray(x[b].T)
        own = _own_cols(h)
        in_maps.append(
            {
                "xt": _pack(xt_b.astype(f8), SB),
                "xst8": _pack(xt_b[:, own].astype(f8), SB),
                "xst": _pack(xt_b[:, own].astype(bf16), SB),
                "wq": Wq8c,
                "wk": Wk8c,
                "wv": Wvb,
                "bk2": bk2,
                "bq2": bq2,
                "bv2": bv2,
                "maskbias": masks[h],
            }
        )
    return in_maps


def _assemble_output(x, results):
    x = np.asarray(x, np.float32)
    out = np.empty((x.shape[0], T, C + VD), np.float32)
    out[:, :, :C] = x
    for b in range(x.shape[0]):
        out[b, :, C:] = results[2 * b]["read_out"].astype(np.float32) + results[
            2 * b + 1
        ]["read_out"].astype(np.float32)
    return out


_NC_CACHE = None


def _build():
    global _NC_CACHE
    if _NC_CACHE is None:
        _NC_CACHE = _build_nc(num_devices=8)
    return _NC_CACHE


def kernel(x, Wq, bq, Wk, bk, Wv, bv):
    from concourse.bass_utils import run_bass_kernel_spmd

    nc = _build()
    in_maps = _make_in_maps(x, Wq, bq, Wk, bk, Wv, bv)
    res = run_bass_kernel_spmd(nc, in_maps, core_ids=list(range(8)))
    return _assemble_output(x, res.results)
